# revision 20
# baseline (speedup 1.0000x reference)
"""NetTGCN forward pass on 8 Trainium2 NeuronCores (Bass/Tile).

Batch-parallel design, zero collectives until the fc head:
  Each core owns 4 batches. Layer-1 channels = 4 batches x 32 taps = 128 =
  exactly the SBUF partition width, so the full Chebyshev recurrence on the
  4096-node graph runs locally per core: state kept in SBUF in both
  [ch, node] (recurrence/contract) and node-major lhsT form (matmul
  stationary). The dense operator 2A^T (bf16, 33.5 MB) is split: 14 of 32
  contract row-tiles stay SBUF-resident, the other 18 are streamed from HBM
  per 512-column output slice (2.4 MB contiguous DMAs, hidden under the
  matmuls). Per Chebyshev term: 256 matmuls of [128x128]@[128x512] (~99% PE
  eff), 32 PE transposes to rebuild the lhsT form, and an inline W1[k]
  contraction into the fp32 h1 accumulator.
  The FFT is folded into W1 on the host (real(FFT(x)) = x @ Ccos commutes
  with the graph operator).
  Layer 2 (1024-node graph) is identical in structure with the 2 MB
  operator fully resident.
  Head: features are exchanged with one 8-rank AllToAll so each core
  contracts its 8192-row slice of fc1_w for all 32 batches; partial z is
  ReduceScattered (each core gets its own 4 batches), fc2 + log_softmax run
  locally, and the host concatenates per-core outputs.

States are bf16 throughout (validated on host: final rel err 6.4e-3 vs
6.1e-3 for fp32 states); accumulators (h1/h2/psum) are fp32.
"""

import sys

if "/opt/trn_rl_repo" not in sys.path:
    sys.path.insert(0, "/opt/trn_rl_repo")

import numpy as np
import ml_dtypes

import concourse.bacc as bacc
import concourse.mybir as mybir
import concourse.bass_utils as _bu
from concourse.bass_utils import run_bass_kernel_spmd
from concourse.tile import TileContext
from concourse.masks import make_identity

_bu.upload_artifacts = lambda tmpdir: f"file://{tmpdir}"  # no bucket in sandbox

F32 = mybir.dt.float32
BF16 = mybir.dt.bfloat16
AX = mybir.AxisListType
ALU = mybir.AluOpType
ACT = mybir.ActivationFunctionType

B, N0, T, K = 32, 4096, 30, 25
G1, G2, D, C = 32, 64, 512, 10
N2 = N0 // 4
NCORES = 8
BL = B // NCORES       # 4 batches per core
TP = 32                # taps padded 30 -> 32
CH = BL * TP           # 128 layer-1 channels = partition width
NT0 = N0 // 128        # 32 contract tiles (layer 1)
NRES = 14              # operator row-tiles resident in SBUF
NSTR = NT0 - NRES      # 19 streamed row-tiles
JRES = NRES * 128
SL = 512               # output slice width
NSL = N0 // SL         # 8 slices per term
NT2 = N2 // 128        # 8 contract tiles (layer 2)
FBLK = (N2 * G2) // NCORES  # 8192 fc1 contraction rows per core

G8 = [list(range(NCORES))]


def _b16(a):
    return np.ascontiguousarray(a.astype(ml_dtypes.bfloat16))


def _dense_adj(edge_index, n):
    row = edge_index[0].astype(np.int64)
    col = edge_index[1].astype(np.int64)
    deg = np.zeros(n, np.float32)
    np.add.at(deg, row, 1.0)
    dis = np.where(deg > 0, 1.0 / np.sqrt(np.maximum(deg, 1.0)), 0.0).astype(np.float32)
    w = (-dis[row] * dis[col]).astype(np.float32)
    a = np.zeros((n, n), np.float32)
    np.add.at(a, (row, col), w)
    return a


def build_program(dbg=False):
    nc = bacc.Bacc("TRN2", target_bir_lowering=False, debug=False,
                   num_devices=NCORES)

    at_res_in = nc.dram_tensor("at_res", [128 * NRES, N0], BF16, kind="ExternalInput")
    at_str_in = nc.dram_tensor("at_str", [NSL * 128 * NSTR, SL], BF16, kind="ExternalInput")
    x_cn_in = nc.dram_tensor("x_cn", [128, N0], BF16, kind="ExternalInput")
    x_lt_in = nc.dram_tensor("x_lt", [128 * NT0, CH], BF16, kind="ExternalInput")
    w1_in = nc.dram_tensor("w1a", [128, K * G1], BF16, kind="ExternalInput")
    b1_in = nc.dram_tensor("b1v", [128, 1], F32, kind="ExternalInput")
    a2t_in = nc.dram_tensor("a2t", [128 * NT2, N2], BF16, kind="ExternalInput")
    w2_in = nc.dram_tensor("w2a", [128, K * 2 * G1], BF16, kind="ExternalInput")
    b2_in = nc.dram_tensor("b2v", [128, 2], F32, kind="ExternalInput")
    fc1w_in = nc.dram_tensor("fc1w", [128 * (FBLK // 128), D], BF16, kind="ExternalInput")
    fc1b_in = nc.dram_tensor("fc1b", [BL, D], F32, kind="ExternalInput")
    fc2w_in = nc.dram_tensor("fc2w", [D, C], BF16, kind="ExternalInput")
    fc2b_in = nc.dram_tensor("fc2b", [BL, C], F32, kind="ExternalInput")

    out_t = nc.dram_tensor("out", [BL, C], F32, kind="ExternalOutput")
    if dbg:
        h1_dbg = nc.dram_tensor("h1_dbg", [128, N0], F32, kind="ExternalOutput")
        h1p_dbg = nc.dram_tensor("h1p_dbg", [128, N2], F32, kind="ExternalOutput")
        h2_dbg = nc.dram_tensor("h2_dbg", [128, 2 * N2], F32, kind="ExternalOutput")
        z_dbg = nc.dram_tensor("z_dbg", [BL, D], F32, kind="ExternalOutput")

    cch_in = nc.dram_tensor("cch_in", [NCORES * 128 * 64, BL], BF16)
    cch_out = nc.dram_tensor("cch_out", [NCORES * 128 * 64, BL], BF16)
    ccz_in = nc.dram_tensor("ccz_in", [B, D], F32)
    ccz_out = nc.dram_tensor("ccz_out", [BL, D], F32)

    with TileContext(nc) as tc:
        with tc.tile_pool(name="const", bufs=1) as cpool:
            identb = cpool.tile([128, 128], BF16)
            make_identity(nc, identb[:])
            identf = cpool.tile([128, 128], F32)
            make_identity(nc, identf[:])
            h1_sb = cpool.tile([128, N0], F32)
            h1p = cpool.tile([128, N2], F32)

            # ======================= LAYER 1 =======================
            with tc.tile_pool(name="l1a", bufs=1) as l1a, \
                 tc.tile_pool(name="l1s", bufs=3) as l1s, \
                 tc.tile_pool(name="l1st", bufs=1) as l1st, \
                 tc.tile_pool(name="ps_y", bufs=2, space="PSUM") as ps_y, \
                 tc.tile_pool(name="ps_tr", bufs=4, space="PSUM") as ps_tr, \
                 tc.tile_pool(name="ps_h", bufs=2, space="PSUM") as ps_h:

                # small inputs first: the DMA rings are FIFO, so the x /
                # weight loads must not queue behind 13 MB of operator tiles
                w1a = l1a.tile([128, K, G1], BF16)
                nc.sync.dma_start(w1a[:], w1_in.ap().rearrange("p (k g) -> p k g", k=K))
                b1v = l1a.tile([128, 1], F32)
                nc.sync.dma_start(b1v[:], b1_in.ap())
                cn = [l1st.tile([128, N0], BF16, name=f"cn{i}", tag=f"cn{i}")
                      for i in range(2)]
                lt = [l1st.tile([128, NT0, CH], BF16, name=f"lt{i}", tag=f"lt{i}")
                      for i in range(2)]
                nc.sync.dma_start(cn[0][:], x_cn_in.ap())
                nc.sync.dma_start(lt[0][:],
                                  x_lt_in.ap().rearrange("(p t) c -> p t c", t=NT0))

                at_res = l1a.tile([128, NRES, N0], BF16)
                at_res_v = at_res_in.ap().rearrange("(p t) n -> p t n", t=NRES)
                for t in range(NRES):
                    nc.sync.dma_start(at_res[:, t, :], at_res_v[:, t, :])

                at_str_v = at_str_in.ap().rearrange(
                    "(g p t) n -> g p t n", g=NSL, p=128)

                def contract1(src_cn, kk, g, first):
                    sl = slice(SL * g, SL * (g + 1))
                    hp = ps_h.tile([128, SL], F32, tag="hp", name=f"hp{kk}_{g}")
                    for bb in range(4):
                        nc.tensor.matmul(
                            hp[32 * bb:32 * (bb + 1), :],
                            w1a[32 * bb:32 * (bb + 1), kk, :],
                            src_cn[32 * bb:32 * (bb + 1), sl],
                            start=True, stop=True,
                            tile_position=(32 * bb, 32 * bb))
                    if first:
                        nc.vector.tensor_copy(h1_sb[:, sl], hp[:])
                    else:
                        nc.vector.tensor_tensor(h1_sb[:, sl], h1_sb[:, sl],
                                                hp[:], ALU.add)

                for g in range(NSL):
                    contract1(cn[0], 0, g, True)

                def epi1(k, g):
                    """Transposes + h1 contract for slice (k, g); emitted
                    after the NEXT slice's matmuls so PE never stalls on
                    the DVE recurrence."""
                    new_cn = cn[k % 2]
                    new_lt = lt[k % 2]
                    for bq in range(4):
                        trp = ps_tr.tile([128, 128], BF16, tag="trp",
                                         name=f"trp{k}_{g}_{bq}")
                        nc.tensor.transpose(
                            trp[:],
                            new_cn[:, SL * g + 128 * bq:SL * g + 128 * (bq + 1)],
                            identb[:])
                        nc.vector.tensor_copy(new_lt[:, 4 * g + bq, :], trp[:])
                    contract1(new_cn, k, g, False)

                for k in range(1, K):
                    cur_lt = lt[(k - 1) % 2]
                    # in-place ring: tx_k overwrites tx_{k-2} slice by slice
                    new_cn = cn[k % 2]
                    prev_cn = new_cn if k >= 2 else None
                    for g in range(NSL):
                        sl = slice(SL * g, SL * (g + 1))
                        sa = l1s.tile([128, 9, SL], BF16, tag="sa",
                                      name=f"sa{k}_{g}")
                        sb = l1s.tile([128, 9, SL], BF16, tag="sa",
                                      name=f"sb{k}_{g}")
                        nc.sync.dma_start(sa[:], at_str_v[g][:, 0:9, :])
                        nc.sync.dma_start(sb[:], at_str_v[g][:, 9:18, :])
                        yp = ps_y.tile([128, SL], F32, tag="yp", name=f"yp{k}_{g}")
                        for jt in range(NRES):
                            nc.tensor.matmul(yp[:], cur_lt[:, jt, :],
                                             at_res[:, jt, sl],
                                             start=(jt == 0), stop=False)
                        for t in range(9):
                            nc.tensor.matmul(yp[:], cur_lt[:, NRES + t, :],
                                             sa[:, t, :],
                                             start=False, stop=False)
                        for t in range(9):
                            nc.tensor.matmul(yp[:], cur_lt[:, NRES + 9 + t, :],
                                             sb[:, t, :],
                                             start=False, stop=(t == 8))
                        if k == 1:
                            nc.vector.tensor_scalar_mul(new_cn[:, sl], yp[:], 0.5)
                        else:
                            nc.vector.tensor_tensor(new_cn[:, sl], yp[:],
                                                    prev_cn[:, sl], ALU.subtract)
                        if g > 0:
                            epi1(k, g - 1)
                    epi1(k, NSL - 1)

                # bias + relu + maxpool4 along nodes
                for q in range(4):
                    nc.scalar.activation(h1_sb[:, 1024 * q:1024 * (q + 1)],
                                         h1_sb[:, 1024 * q:1024 * (q + 1)],
                                         ACT.Relu, bias=b1v[:])
                if dbg:
                    nc.sync.dma_start(h1_dbg.ap(), h1_sb[:])
                h4 = h1_sb[:].rearrange("p (n f) -> p n f", f=4)
                nc.vector.tensor_tensor(h1p[:], h4[:, :, 0], h4[:, :, 1], ALU.max)
                nc.vector.tensor_tensor(h1p[:], h1p[:], h4[:, :, 2], ALU.max)
                nc.vector.tensor_tensor(h1p[:], h1p[:], h4[:, :, 3], ALU.max)
                if dbg:
                    nc.sync.dma_start(h1p_dbg.ap(), h1p[:])

            # ======================= LAYER 2 =======================
            with tc.tile_pool(name="l2", bufs=1) as l2, \
                 tc.tile_pool(name="l2st", bufs=1) as l2st, \
                 tc.tile_pool(name="ps2t", bufs=2, space="PSUM") as ps2t:

                a2t = l2.tile([128, NT2, N2], BF16)
                a2t_v = a2t_in.ap().rearrange("(p t) n -> p t n", t=NT2)
                for t in range(NT2):
                    nc.sync.dma_start(a2t[:, t, :], a2t_v[:, t, :])
                w2a = l2.tile([128, K, 2, G1], BF16)
                nc.sync.dma_start(
                    w2a[:], w2_in.ap().rearrange("p (k h g) -> p k h g", k=K, h=2))
                b2v = l2.tile([128, 2], F32)
                nc.sync.dma_start(b2v[:], b2_in.ap())
                fcw = l2.tile([128, FBLK // 128, D], BF16)
                fcw_v = fc1w_in.ap().rearrange("(p t) d -> p t d", t=FBLK // 128)
                for q in range(8):
                    nc.sync.dma_start(fcw[:, 8 * q:8 * (q + 1), :],
                                      fcw_v[:, 8 * q:8 * (q + 1), :])

                cn2 = [l2st.tile([128, N2], BF16, name=f"cn2_{i}", tag=f"cn2_{i}")
                       for i in range(3)]
                lt2 = [l2st.tile([128, NT2, CH], BF16, name=f"lt2_{i}",
                                 tag=f"lt2_{i}") for i in range(2)]
                h2r = l2.tile([128, 2, N2], F32)
                ft = l2.tile([128, NT2, G2, BL], BF16)

                with tc.tile_pool(name="ps2y", bufs=2, space="PSUM") as ps2y, \
                     tc.tile_pool(name="ps2h", bufs=1, space="PSUM") as ps2h:
                    # h2 accumulates in PSUM across all K terms: 4 banks,
                    # start at k=0, stop at k=K-1, relu reads PSUM directly.
                    h2ps = [[ps2h.tile([128, SL], F32, tag=f"h2ps_{hh}_{g}",
                                       name=f"h2ps_{hh}_{g}")
                             for g in range(2)] for hh in range(2)]

                    nc.vector.tensor_copy(cn2[0][:], h1p[:])
                    for nt in range(NT2):
                        trp = ps2t.tile([128, 128], BF16, tag="tr2",
                                        name=f"tr2i_{nt}")
                        nc.tensor.transpose(
                            trp[:], cn2[0][:, 128 * nt:128 * (nt + 1)], identb[:])
                        nc.vector.tensor_copy(lt2[0][:, nt, :], trp[:])

                    def contract2(src_cn, kk, g):
                        sl = slice(SL * g, SL * (g + 1))
                        for hh in range(2):
                            for bb in range(4):
                                nc.tensor.matmul(
                                    h2ps[hh][g][32 * bb:32 * (bb + 1), :],
                                    w2a[32 * bb:32 * (bb + 1), kk, hh, :],
                                    src_cn[32 * bb:32 * (bb + 1), sl],
                                    start=(kk == 0), stop=(kk == K - 1),
                                    tile_position=(32 * bb, 32 * bb))

                    def epi2(k, g):
                        new_cn2 = cn2[k % 3]
                        new_lt2 = lt2[k % 2]
                        for bq in range(4):
                            trp = ps2t.tile([128, 128], BF16, tag="tr2",
                                            name=f"tr2_{k}_{g}_{bq}")
                            nc.tensor.transpose(
                                trp[:],
                                new_cn2[:, SL * g + 128 * bq:SL * g + 128 * (bq + 1)],
                                identb[:])
                            nc.vector.tensor_copy(new_lt2[:, 4 * g + bq, :], trp[:])
                        contract2(new_cn2, k, g)

                    contract2(cn2[0], 0, 0)
                    contract2(cn2[0], 0, 1)
                    for k in range(1, K):
                        cur_lt2 = lt2[(k - 1) % 2]
                        new_cn2 = cn2[k % 3]
                        prev_cn2 = cn2[(k - 2) % 3] if k >= 2 else None
                        for g in range(2):
                            sl = slice(SL * g, SL * (g + 1))
                            y2 = ps2y.tile([128, SL], F32, tag="y2",
                                           name=f"y2_{k}_{g}")
                            for jt in range(NT2):
                                nc.tensor.matmul(y2[:], cur_lt2[:, jt, :],
                                                 a2t[:, jt, sl],
                                                 start=(jt == 0),
                                                 stop=(jt == NT2 - 1))
                            if k == 1:
                                nc.vector.tensor_scalar_mul(new_cn2[:, sl],
                                                            y2[:], 0.5)
                            else:
                                nc.vector.tensor_tensor(new_cn2[:, sl], y2[:],
                                                        prev_cn2[:, sl],
                                                        ALU.subtract)
                            if g > 0:
                                epi2(k, g - 1)
                        epi2(k, 1)

                    # ================== HEAD (part 1) ==================
                    for hh in range(2):
                        for g in range(2):
                            sl = slice(SL * g, SL * (g + 1))
                            nc.scalar.activation(h2r[:, hh, sl], h2ps[hh][g][:],
                                                 ACT.Relu, bias=b2v[:, hh:hh + 1])
                    if dbg:
                        nc.sync.dma_start(
                            h2_dbg.ap().rearrange("p (h n) -> p h n", h=2),
                            h2r[:])

                    # features to f-major: ft[n2l, nt, g2, b]
                    for hh in range(2):
                        for nt in range(NT2):
                            trp = ps2t.tile([128, 128], F32, tag="tr2",
                                            name=f"trh_{hh}_{nt}")
                            nc.tensor.transpose(
                                trp[:], h2r[:, hh, 128 * nt:128 * (nt + 1)],
                                identf[:])
                            nc.vector.tensor_copy(
                                out=ft[:, nt, 32 * hh:32 * (hh + 1), :],
                                in_=trp[:].rearrange("p (b g) -> p g b", b=BL))
                    nc.sync.dma_start(
                        cch_in.ap().rearrange("(j nl g) b -> nl j (g b)",
                                              j=NCORES, nl=128),
                        ft[:])
                    nc.gpsimd.collective_compute(
                        "AllToAll", ALU.bypass, replica_groups=G8,
                        ins=[cch_in.ap()], outs=[cch_out.ap()])

                # ================== HEAD (part 2) ==================
                # fc1 partial: z[32, D] for my f-slice. flt is
                # [p=n2l, r, q=g2, b] so each per-rank DMA from cch_out is
                # contiguous on both sides; the matmul stationary reads the
                # strided [p, (r, b)] AP per g2.
                with tc.tile_pool(name="psz", bufs=1, space="PSUM") as psz, \
                     tc.tile_pool(name="psl", bufs=1, space="PSUM") as psl:
                    head_tail(nc, tc, l2, ps2t, psz, psl, identf,
                              cch_out, ccz_in, ccz_out, fcw,
                              fc1b_in, fc2w_in, fc2b_in, out_t,
                              z_dbg if dbg else None)

    nc.compile()
    return nc


def head_tail(nc, tc, l2, ps2t, psz, psl, identf, cch_out, ccz_in, ccz_out,
              fcw, fc1b_in, fc2w_in, fc2b_in, out_t, z_dbg):
    """fc1 partial + ReduceScatter + fc2 + log_softmax."""
    if True:
        if True:
            if True:
                flt = l2.tile([128, NCORES, G2, BL], BF16)
                cch_v = cch_out.ap().rearrange(
                    "(r p q) b -> r p q b", r=NCORES, p=128)
                for r in range(NCORES):
                    nc.sync.dma_start(flt[:, r, :, :], cch_v[r])
                flt2 = l2.tile([128, G2, B], BF16)
                nc.vector.tensor_copy(
                    out=flt2[:].rearrange("p q (r b) -> p q r b", r=NCORES),
                    in_=flt[:].rearrange("p r q b -> p q r b"))
                zps = psz.tile([32, D], F32)
                for kt in range(G2):
                    nc.tensor.matmul(zps[:], flt2[:, kt, :], fcw[:, kt, :],
                                     start=(kt == 0), stop=(kt == G2 - 1))
                zblk = l2.tile([32, D], F32)
                nc.vector.tensor_copy(zblk[:], zps[:])
                nc.sync.dma_start(ccz_in.ap(), zblk[:])
                nc.gpsimd.collective_compute(
                    "ReduceScatter", ALU.add, replica_groups=G8,
                    ins=[ccz_in.ap()], outs=[ccz_out.ap()])
                zfull = l2.tile([BL, D], F32)
                nc.sync.dma_start(zfull[:], ccz_out.ap())
                zb = l2.tile([BL, D], F32)
                nc.sync.dma_start(zb[:], fc1b_in.ap())
                nc.vector.tensor_tensor(zfull[:], zfull[:], zb[:], ALU.add)
                zr = l2.tile([BL, D], F32)
                nc.scalar.activation(zr[:], zfull[:], ACT.Relu)
                if z_dbg is not None:
                    nc.sync.dma_start(z_dbg.ap(), zr[:])

                # fc2 + log_softmax on my 4 batches
                f2w = l2.tile([128, 4, C], BF16)
                nc.sync.dma_start(f2w[:],
                                  fc2w_in.ap().rearrange("(t p) c -> p t c", p=128))
                lps = psl.tile([BL, C], F32)
                for t4 in range(4):
                    ztp = ps2t.tile([128, BL], F32, tag="tr2", name=f"zt_{t4}")
                    nc.tensor.transpose(ztp[:], zr[:, 128 * t4:128 * (t4 + 1)],
                                        identf[:BL, :BL])
                    zts = l2.tile([128, BL], BF16, tag="zts", name=f"zts_{t4}")
                    nc.any.tensor_copy(out=zts[:], in_=ztp[:])
                    nc.tensor.matmul(lps[:], zts[:], f2w[:, t4, :],
                                     start=(t4 == 0), stop=(t4 == 3))
                logits = l2.tile([BL, C], F32)
                f2b = l2.tile([BL, C], F32)
                nc.sync.dma_start(f2b[:], fc2b_in.ap())
                nc.vector.tensor_tensor(logits[:], lps[:], f2b[:], ALU.add)

                mx = l2.tile([BL, 1], F32)
                nc.vector.tensor_reduce(mx[:], logits[:], axis=AX.X, op=ALU.max)
                sh = l2.tile([BL, C], F32)
                nc.vector.tensor_tensor(sh[:], logits[:],
                                        mx[:].to_broadcast((BL, C)), ALU.subtract)
                ex = l2.tile([BL, C], F32)
                nc.scalar.activation(ex[:], sh[:], ACT.Exp)
                sm = l2.tile([BL, 1], F32)
                nc.vector.tensor_reduce(sm[:], ex[:], axis=AX.X, op=ALU.add)
                lg = l2.tile([BL, 1], F32)
                nc.scalar.activation(lg[:], sm[:], ACT.Ln)
                res = l2.tile([BL, C], F32)
                nc.vector.tensor_tensor(res[:], sh[:],
                                        lg[:].to_broadcast((BL, C)), ALU.subtract)
                nc.sync.dma_start(out_t.ap(), res[:])


def _identity_cos():
    t = np.arange(T)
    f = np.arange(T)
    return np.cos(2.0 * np.pi * np.outer(t, f) / T).astype(np.float32)


def make_inputs(x, edge_index0, edge_index2, W1, b1, W2, b2,
                fc1_w, fc1_b, fc2_w, fc2_b):
    """Build the 8 per-core input maps."""
    A0 = _dense_adj(np.asarray(edge_index0), N0)
    A2 = _dense_adj(np.asarray(edge_index2), N2)
    At1 = np.ascontiguousarray((2.0 * A0).T)   # [j, i] = 2*A0[i, j]
    At2 = np.ascontiguousarray((2.0 * A2).T)

    # resident: rows p*NRES+t = At1 row t*128+p
    at_res = _b16(At1[:JRES].reshape(NRES, 128, N0)
                  .transpose(1, 0, 2).reshape(128 * NRES, N0))
    s = At1[JRES:].reshape(NSTR, 128, N0).transpose(1, 0, 2)  # [p, t, n]
    at_str = np.stack([s[:, :, SL * g:SL * (g + 1)] for g in range(NSL)], 0)
    at_str = _b16(at_str.reshape(NSL * 128 * NSTR, SL))
    a2t = _b16(At2.reshape(NT2, 128, N2).transpose(1, 0, 2).reshape(128 * NT2, N2))

    Ccos = _identity_cos()
    W1e = np.einsum("tf,kfg->ktg", Ccos, np.asarray(W1, np.float32))  # [K, 30, G1]
    w1a = np.zeros((128, K, G1), np.float32)
    for bb in range(4):
        w1a[32 * bb:32 * bb + 30] = W1e.transpose(1, 0, 2)
    w1a = _b16(w1a.reshape(128, K * G1))

    W2f = np.asarray(W2, np.float32)       # [K, G1, G2]
    w2a = np.zeros((128, K, 2, G1), np.float32)
    for bb in range(4):
        for hh in range(2):
            w2a[32 * bb:32 * bb + 32, :, hh, :] = \
                W2f[:, :, 32 * hh:32 * hh + 32].transpose(1, 0, 2)
    w2a = _b16(w2a.reshape(128, K * 2 * G1))

    b1v = np.tile(np.asarray(b1, np.float32), 4).reshape(128, 1)
    b2f = np.asarray(b2, np.float32)
    b2v = np.stack([np.tile(b2f[:32], 4), np.tile(b2f[32:], 4)], 1).astype(np.float32)

    fc1b = np.tile(np.asarray(fc1_b, np.float32)[None, :], (BL, 1))
    fc2b = np.tile(np.asarray(fc2_b, np.float32)[None, :], (BL, 1))
    fc2w = _b16(np.asarray(fc2_w, np.float32))

    xf = np.asarray(x, np.float32)          # [B, N0, T]
    fc1wf = np.asarray(fc1_w, np.float32)   # [N2*G2, D]

    ins = []
    for core in range(NCORES):
        xs = xf[BL * core:BL * (core + 1)]          # [4, N0, 30]
        x_cn = np.zeros((BL, TP, N0), np.float32)
        x_cn[:, :T] = xs.transpose(0, 2, 1)
        x_cn = x_cn.reshape(CH, N0)                 # row = b*32 + t
        x_lt = x_cn.T.reshape(NT0, 128, CH).transpose(1, 0, 2).reshape(128 * NT0, CH)
        # fcw[p=n2l, kt=g2, d] = fc1_w[8192*core + n2l*64 + g2, d]: the
        # natural row-major order of the core's slice, no interleave.
        fc1w_r = fc1wf[FBLK * core:FBLK * (core + 1)]
        ins.append({
            "at_res": at_res, "at_str": at_str, "a2t": a2t,
            "x_cn": _b16(x_cn), "x_lt": _b16(x_lt),
            "w1a": w1a, "w2a": w2a, "b1v": b1v, "b2v": b2v,
            "fc1w": _b16(fc1w_r), "fc1b": fc1b,
            "fc2w": fc2w, "fc2b": fc2b,
        })
    return ins


_CACHED = {}


def kernel(**inputs):
    if "nc" not in _CACHED:
        _CACHED["nc"] = build_program(dbg=False)
    nc = _CACHED["nc"]
    ins = make_inputs(**inputs)
    res = run_bass_kernel_spmd(nc, ins, core_ids=list(range(NCORES)))
    out = np.zeros((B, C), np.float32)
    for core in range(NCORES):
        out[BL * core:BL * (core + 1)] = res.results[core]["out"]
    return out


# revision 21
# speedup vs baseline: 1.0296x; 1.0296x over previous
"""NetTGCN forward pass on 8 Trainium2 NeuronCores (Bass/Tile).

Batch-parallel design, zero collectives until the fc head:
  Each core owns 4 batches. Layer-1 channels = 4 batches x 32 taps = 128 =
  exactly the SBUF partition width, so the full Chebyshev recurrence on the
  4096-node graph runs locally per core: state kept in SBUF in both
  [ch, node] (recurrence/contract) and node-major lhsT form (matmul
  stationary). The dense operator 2A^T (bf16, 33.5 MB) is split: 14 of 32
  contract row-tiles stay SBUF-resident, the other 18 are streamed from HBM
  per 512-column output slice (2.4 MB contiguous DMAs, hidden under the
  matmuls). Per Chebyshev term: 256 matmuls of [128x128]@[128x512] (~99% PE
  eff), 32 PE transposes to rebuild the lhsT form, and an inline W1[k]
  contraction into the fp32 h1 accumulator.
  The FFT is folded into W1 on the host (real(FFT(x)) = x @ Ccos commutes
  with the graph operator).
  Layer 2 (1024-node graph) is identical in structure with the 2 MB
  operator fully resident.
  Head: features are exchanged with one 8-rank AllToAll so each core
  contracts its 8192-row slice of fc1_w for all 32 batches; partial z is
  ReduceScattered (each core gets its own 4 batches), fc2 + log_softmax run
  locally, and the host concatenates per-core outputs.

States are bf16 throughout (validated on host: final rel err 6.4e-3 vs
6.1e-3 for fp32 states); accumulators (h1/h2/psum) are fp32.
"""

import sys

if "/opt/trn_rl_repo" not in sys.path:
    sys.path.insert(0, "/opt/trn_rl_repo")

import numpy as np
import ml_dtypes

import concourse.bacc as bacc
import concourse.mybir as mybir
import concourse.bass_utils as _bu
from concourse.bass_utils import run_bass_kernel_spmd
from concourse.tile import TileContext
from concourse.masks import make_identity

_bu.upload_artifacts = lambda tmpdir: f"file://{tmpdir}"  # no bucket in sandbox

F32 = mybir.dt.float32
BF16 = mybir.dt.bfloat16
AX = mybir.AxisListType
ALU = mybir.AluOpType
ACT = mybir.ActivationFunctionType

B, N0, T, K = 32, 4096, 30, 25
G1, G2, D, C = 32, 64, 512, 10
N2 = N0 // 4
NCORES = 8
BL = B // NCORES       # 4 batches per core
TP = 32                # taps padded 30 -> 32
CH = BL * TP           # 128 layer-1 channels = partition width
NT0 = N0 // 128        # 32 contract tiles (layer 1)
NRES = 14              # operator row-tiles resident in SBUF
NSTR = NT0 - NRES      # 19 streamed row-tiles
JRES = NRES * 128
SL = 512               # output slice width
NSL = N0 // SL         # 8 slices per term
NT2 = N2 // 128        # 8 contract tiles (layer 2)
FBLK = (N2 * G2) // NCORES  # 8192 fc1 contraction rows per core

G8 = [list(range(NCORES))]


def _b16(a):
    return np.ascontiguousarray(a.astype(ml_dtypes.bfloat16))


def _dense_adj(edge_index, n):
    row = edge_index[0].astype(np.int64)
    col = edge_index[1].astype(np.int64)
    deg = np.zeros(n, np.float32)
    np.add.at(deg, row, 1.0)
    dis = np.where(deg > 0, 1.0 / np.sqrt(np.maximum(deg, 1.0)), 0.0).astype(np.float32)
    w = (-dis[row] * dis[col]).astype(np.float32)
    a = np.zeros((n, n), np.float32)
    np.add.at(a, (row, col), w)
    return a


def build_program(dbg=False):
    nc = bacc.Bacc("TRN2", target_bir_lowering=False, debug=False,
                   num_devices=NCORES)

    at_res_in = nc.dram_tensor("at_res", [128 * NRES, N0], BF16, kind="ExternalInput")
    at_str_in = nc.dram_tensor("at_str", [NSL * 128 * NSTR, SL], BF16, kind="ExternalInput")
    x_cn_in = nc.dram_tensor("x_cn", [128, N0], BF16, kind="ExternalInput")
    x_lt_in = nc.dram_tensor("x_lt", [128 * NT0, CH], BF16, kind="ExternalInput")
    w1_in = nc.dram_tensor("w1a", [128, K * G1], BF16, kind="ExternalInput")
    b1_in = nc.dram_tensor("b1v", [128, 1], F32, kind="ExternalInput")
    a2t_in = nc.dram_tensor("a2t", [128 * NT2, N2], BF16, kind="ExternalInput")
    w2_in = nc.dram_tensor("w2a", [128, K * 2 * G1], BF16, kind="ExternalInput")
    b2_in = nc.dram_tensor("b2v", [128, 2], F32, kind="ExternalInput")
    fc1w_in = nc.dram_tensor("fc1w", [128 * (FBLK // 128), D], BF16, kind="ExternalInput")
    fc1b_in = nc.dram_tensor("fc1b", [BL, D], F32, kind="ExternalInput")
    fc2w_in = nc.dram_tensor("fc2w", [D, C], BF16, kind="ExternalInput")
    fc2b_in = nc.dram_tensor("fc2b", [BL, C], F32, kind="ExternalInput")

    out_t = nc.dram_tensor("out", [BL, C], F32, kind="ExternalOutput")
    if dbg:
        h1_dbg = nc.dram_tensor("h1_dbg", [128, N0], F32, kind="ExternalOutput")
        h1p_dbg = nc.dram_tensor("h1p_dbg", [128, N2], F32, kind="ExternalOutput")
        h2_dbg = nc.dram_tensor("h2_dbg", [128, 2 * N2], F32, kind="ExternalOutput")
        z_dbg = nc.dram_tensor("z_dbg", [BL, D], F32, kind="ExternalOutput")

    cch_in = nc.dram_tensor("cch_in", [NCORES * 128 * 64, BL], BF16)
    cch_out = nc.dram_tensor("cch_out", [NCORES * 128 * 64, BL], BF16)
    ccz_in = nc.dram_tensor("ccz_in", [B, D], F32)
    ccz_out = nc.dram_tensor("ccz_out", [BL, D], F32)

    with TileContext(nc) as tc:
        with tc.tile_pool(name="const", bufs=1) as cpool:
            identb = cpool.tile([128, 128], BF16)
            make_identity(nc, identb[:])
            identf = cpool.tile([128, 128], F32)
            make_identity(nc, identf[:])
            h1_sb = cpool.tile([128, N0], F32)
            h1p = cpool.tile([128, N2], F32)

            # ======================= LAYER 1 =======================
            with tc.tile_pool(name="l1a", bufs=1) as l1a, \
                 tc.tile_pool(name="l1s", bufs=3) as l1s, \
                 tc.tile_pool(name="l1st", bufs=1) as l1st, \
                 tc.tile_pool(name="ps_y", bufs=2, space="PSUM") as ps_y, \
                 tc.tile_pool(name="ps_tr", bufs=4, space="PSUM") as ps_tr, \
                 tc.tile_pool(name="ps_h", bufs=2, space="PSUM") as ps_h:

                # small inputs first: the DMA rings are FIFO, so the x /
                # weight loads must not queue behind 13 MB of operator tiles
                w1a = l1a.tile([128, K, G1], BF16)
                nc.sync.dma_start(w1a[:], w1_in.ap().rearrange("p (k g) -> p k g", k=K))
                b1v = l1a.tile([128, 1], F32)
                nc.sync.dma_start(b1v[:], b1_in.ap())
                cn = [l1st.tile([128, N0], BF16, name=f"cn{i}", tag=f"cn{i}")
                      for i in range(2)]
                lt = [l1st.tile([128, NT0, CH], BF16, name=f"lt{i}", tag=f"lt{i}")
                      for i in range(2)]
                nc.sync.dma_start(cn[0][:], x_cn_in.ap())
                nc.sync.dma_start(lt[0][:],
                                  x_lt_in.ap().rearrange("(p t) c -> p t c", t=NT0))

                at_res = l1a.tile([128, NRES, N0], BF16)
                at_res_v = at_res_in.ap().rearrange("(p t) n -> p t n", t=NRES)
                for t in range(NRES):
                    nc.sync.dma_start(at_res[:, t, :], at_res_v[:, t, :])

                at_str_v = at_str_in.ap().rearrange(
                    "(g p t) n -> g p t n", g=NSL, p=128)

                def contract1(src_cn, kk, g, first):
                    sl = slice(SL * g, SL * (g + 1))
                    hp = ps_h.tile([128, SL], F32, tag="hp", name=f"hp{kk}_{g}")
                    for bb in range(4):
                        nc.tensor.matmul(
                            hp[32 * bb:32 * (bb + 1), :],
                            w1a[32 * bb:32 * (bb + 1), kk, :],
                            src_cn[32 * bb:32 * (bb + 1), sl],
                            start=True, stop=True,
                            tile_position=(32 * bb, 32 * bb))
                    if first:
                        nc.vector.tensor_copy(h1_sb[:, sl], hp[:])
                    else:
                        nc.vector.tensor_tensor(h1_sb[:, sl], h1_sb[:, sl],
                                                hp[:], ALU.add)

                for g in range(NSL):
                    contract1(cn[0], 0, g, True)

                def epi1(k, g):
                    """Transposes + h1 contract for slice (k, g); emitted
                    after the NEXT slice's matmuls so PE never stalls on
                    the DVE recurrence."""
                    new_cn = cn[k % 2]
                    new_lt = lt[k % 2]
                    for bq in range(4):
                        trp = ps_tr.tile([128, 128], BF16, tag="trp",
                                         name=f"trp{k}_{g}_{bq}")
                        nc.tensor.transpose(
                            trp[:],
                            new_cn[:, SL * g + 128 * bq:SL * g + 128 * (bq + 1)],
                            identb[:])
                        nc.vector.tensor_copy(new_lt[:, 4 * g + bq, :], trp[:])
                    contract1(new_cn, k, g, False)

                pend = None
                for k in range(1, K):
                    cur_lt = lt[(k - 1) % 2]
                    # in-place ring: tx_k overwrites tx_{k-2} slice by slice
                    new_cn = cn[k % 2]
                    prev_cn = new_cn if k >= 2 else None
                    for g in range(NSL):
                        sl = slice(SL * g, SL * (g + 1))
                        sa = l1s.tile([128, 9, SL], BF16, tag="sa",
                                      name=f"sa{k}_{g}")
                        sb = l1s.tile([128, 9, SL], BF16, tag="sa",
                                      name=f"sb{k}_{g}")
                        nc.sync.dma_start(sa[:], at_str_v[g][:, 0:9, :])
                        nc.sync.dma_start(sb[:], at_str_v[g][:, 9:18, :])
                        yp = ps_y.tile([128, SL], F32, tag="yp", name=f"yp{k}_{g}")
                        for jt in range(NRES):
                            nc.tensor.matmul(yp[:], cur_lt[:, jt, :],
                                             at_res[:, jt, sl],
                                             start=(jt == 0), stop=False)
                        # previous slice's epilogue sits inside this MM
                        # group: its lhsT tiles are only read by the later
                        # streamed matmuls (t >= 14), so PE never stalls.
                        if pend is not None:
                            pend()
                            pend = None
                        for t in range(9):
                            nc.tensor.matmul(yp[:], cur_lt[:, NRES + t, :],
                                             sa[:, t, :],
                                             start=False, stop=False)
                        for t in range(9):
                            nc.tensor.matmul(yp[:], cur_lt[:, NRES + 9 + t, :],
                                             sb[:, t, :],
                                             start=False, stop=(t == 8))
                        if k == 1:
                            nc.vector.tensor_scalar_mul(new_cn[:, sl], yp[:], 0.5)
                        else:
                            nc.vector.tensor_tensor(new_cn[:, sl], yp[:],
                                                    prev_cn[:, sl], ALU.subtract)
                        pend = (lambda kk=k, gg=g: epi1(kk, gg))
                pend()
                pend = None

                # bias + relu + maxpool4 along nodes
                for q in range(4):
                    nc.scalar.activation(h1_sb[:, 1024 * q:1024 * (q + 1)],
                                         h1_sb[:, 1024 * q:1024 * (q + 1)],
                                         ACT.Relu, bias=b1v[:])
                if dbg:
                    nc.sync.dma_start(h1_dbg.ap(), h1_sb[:])
                h4 = h1_sb[:].rearrange("p (n f) -> p n f", f=4)
                nc.vector.tensor_tensor(h1p[:], h4[:, :, 0], h4[:, :, 1], ALU.max)
                nc.vector.tensor_tensor(h1p[:], h1p[:], h4[:, :, 2], ALU.max)
                nc.vector.tensor_tensor(h1p[:], h1p[:], h4[:, :, 3], ALU.max)
                if dbg:
                    nc.sync.dma_start(h1p_dbg.ap(), h1p[:])

            # ======================= LAYER 2 =======================
            with tc.tile_pool(name="l2", bufs=1) as l2, \
                 tc.tile_pool(name="l2st", bufs=1) as l2st, \
                 tc.tile_pool(name="ps2t", bufs=2, space="PSUM") as ps2t:

                a2t = l2.tile([128, NT2, N2], BF16)
                a2t_v = a2t_in.ap().rearrange("(p t) n -> p t n", t=NT2)
                for t in range(NT2):
                    nc.sync.dma_start(a2t[:, t, :], a2t_v[:, t, :])
                w2a = l2.tile([128, K, 2, G1], BF16)
                nc.sync.dma_start(
                    w2a[:], w2_in.ap().rearrange("p (k h g) -> p k h g", k=K, h=2))
                b2v = l2.tile([128, 2], F32)
                nc.sync.dma_start(b2v[:], b2_in.ap())
                fcw = l2.tile([128, FBLK // 128, D], BF16)
                fcw_v = fc1w_in.ap().rearrange("(p t) d -> p t d", t=FBLK // 128)
                for q in range(8):
                    nc.sync.dma_start(fcw[:, 8 * q:8 * (q + 1), :],
                                      fcw_v[:, 8 * q:8 * (q + 1), :])

                cn2 = [l2st.tile([128, N2], BF16, name=f"cn2_{i}", tag=f"cn2_{i}")
                       for i in range(3)]
                lt2 = [l2st.tile([128, NT2, CH], BF16, name=f"lt2_{i}",
                                 tag=f"lt2_{i}") for i in range(2)]
                h2r = l2.tile([128, 2, N2], F32)
                ft = l2.tile([128, NT2, G2, BL], BF16)

                with tc.tile_pool(name="ps2y", bufs=2, space="PSUM") as ps2y, \
                     tc.tile_pool(name="ps2h", bufs=1, space="PSUM") as ps2h:
                    # h2 accumulates in PSUM across all K terms: 4 banks,
                    # start at k=0, stop at k=K-1, relu reads PSUM directly.
                    h2ps = [[ps2h.tile([128, SL], F32, tag=f"h2ps_{hh}_{g}",
                                       name=f"h2ps_{hh}_{g}")
                             for g in range(2)] for hh in range(2)]

                    nc.vector.tensor_copy(cn2[0][:], h1p[:])
                    for nt in range(NT2):
                        trp = ps2t.tile([128, 128], BF16, tag="tr2",
                                        name=f"tr2i_{nt}")
                        nc.tensor.transpose(
                            trp[:], cn2[0][:, 128 * nt:128 * (nt + 1)], identb[:])
                        nc.vector.tensor_copy(lt2[0][:, nt, :], trp[:])

                    def contract2(src_cn, kk, g):
                        sl = slice(SL * g, SL * (g + 1))
                        for hh in range(2):
                            for bb in range(4):
                                nc.tensor.matmul(
                                    h2ps[hh][g][32 * bb:32 * (bb + 1), :],
                                    w2a[32 * bb:32 * (bb + 1), kk, hh, :],
                                    src_cn[32 * bb:32 * (bb + 1), sl],
                                    start=(kk == 0), stop=(kk == K - 1),
                                    tile_position=(32 * bb, 32 * bb))

                    def epi2(k, g):
                        new_cn2 = cn2[k % 3]
                        new_lt2 = lt2[k % 2]
                        for bq in range(4):
                            trp = ps2t.tile([128, 128], BF16, tag="tr2",
                                            name=f"tr2_{k}_{g}_{bq}")
                            nc.tensor.transpose(
                                trp[:],
                                new_cn2[:, SL * g + 128 * bq:SL * g + 128 * (bq + 1)],
                                identb[:])
                            nc.vector.tensor_copy(new_lt2[:, 4 * g + bq, :], trp[:])
                        contract2(new_cn2, k, g)

                    contract2(cn2[0], 0, 0)
                    contract2(cn2[0], 0, 1)
                    pend2 = None
                    for k in range(1, K):
                        cur_lt2 = lt2[(k - 1) % 2]
                        new_cn2 = cn2[k % 3]
                        prev_cn2 = cn2[(k - 2) % 3] if k >= 2 else None
                        for g in range(2):
                            sl = slice(SL * g, SL * (g + 1))
                            y2 = ps2y.tile([128, SL], F32, tag="y2",
                                           name=f"y2_{k}_{g}")
                            for jt in range(4):
                                nc.tensor.matmul(y2[:], cur_lt2[:, jt, :],
                                                 a2t[:, jt, sl],
                                                 start=(jt == 0), stop=False)
                            if pend2 is not None:
                                pend2()
                                pend2 = None
                            for jt in range(4, NT2):
                                nc.tensor.matmul(y2[:], cur_lt2[:, jt, :],
                                                 a2t[:, jt, sl],
                                                 start=False,
                                                 stop=(jt == NT2 - 1))
                            if k == 1:
                                nc.vector.tensor_scalar_mul(new_cn2[:, sl],
                                                            y2[:], 0.5)
                            else:
                                nc.vector.tensor_tensor(new_cn2[:, sl], y2[:],
                                                        prev_cn2[:, sl],
                                                        ALU.subtract)
                            pend2 = (lambda kk=k, gg=g: epi2(kk, gg))
                    pend2()
                    pend2 = None

                    # ================== HEAD (part 1) ==================
                    for hh in range(2):
                        for g in range(2):
                            sl = slice(SL * g, SL * (g + 1))
                            nc.scalar.activation(h2r[:, hh, sl], h2ps[hh][g][:],
                                                 ACT.Relu, bias=b2v[:, hh:hh + 1])
                    if dbg:
                        nc.sync.dma_start(
                            h2_dbg.ap().rearrange("p (h n) -> p h n", h=2),
                            h2r[:])

                    # features to f-major: ft[n2l, nt, g2, b]
                    for hh in range(2):
                        for nt in range(NT2):
                            trp = ps2t.tile([128, 128], F32, tag="tr2",
                                            name=f"trh_{hh}_{nt}")
                            nc.tensor.transpose(
                                trp[:], h2r[:, hh, 128 * nt:128 * (nt + 1)],
                                identf[:])
                            nc.vector.tensor_copy(
                                out=ft[:, nt, 32 * hh:32 * (hh + 1), :],
                                in_=trp[:].rearrange("p (b g) -> p g b", b=BL))
                    nc.sync.dma_start(
                        cch_in.ap().rearrange("(j nl g) b -> nl j (g b)",
                                              j=NCORES, nl=128),
                        ft[:])
                    nc.gpsimd.collective_compute(
                        "AllToAll", ALU.bypass, replica_groups=G8,
                        ins=[cch_in.ap()], outs=[cch_out.ap()])

                # ================== HEAD (part 2) ==================
                # fc1 partial: z[32, D] for my f-slice. flt is
                # [p=n2l, r, q=g2, b] so each per-rank DMA from cch_out is
                # contiguous on both sides; the matmul stationary reads the
                # strided [p, (r, b)] AP per g2.
                with tc.tile_pool(name="psz", bufs=1, space="PSUM") as psz, \
                     tc.tile_pool(name="psl", bufs=1, space="PSUM") as psl:
                    head_tail(nc, tc, l2, ps2t, psz, psl, identf,
                              cch_out, ccz_in, ccz_out, fcw,
                              fc1b_in, fc2w_in, fc2b_in, out_t,
                              z_dbg if dbg else None)

    nc.compile()
    return nc


def head_tail(nc, tc, l2, ps2t, psz, psl, identf, cch_out, ccz_in, ccz_out,
              fcw, fc1b_in, fc2w_in, fc2b_in, out_t, z_dbg):
    """fc1 partial + ReduceScatter + fc2 + log_softmax."""
    if True:
        if True:
            if True:
                flt = l2.tile([128, NCORES, G2, BL], BF16)
                cch_v = cch_out.ap().rearrange(
                    "(r p q) b -> r p q b", r=NCORES, p=128)
                for r in range(NCORES):
                    nc.sync.dma_start(flt[:, r, :, :], cch_v[r])
                flt2 = l2.tile([128, G2, B], BF16)
                nc.vector.tensor_copy(
                    out=flt2[:].rearrange("p q (r b) -> p q r b", r=NCORES),
                    in_=flt[:].rearrange("p r q b -> p q r b"))
                zps = psz.tile([32, D], F32)
                for kt in range(G2):
                    nc.tensor.matmul(zps[:], flt2[:, kt, :], fcw[:, kt, :],
                                     start=(kt == 0), stop=(kt == G2 - 1))
                zblk = l2.tile([32, D], F32)
                nc.vector.tensor_copy(zblk[:], zps[:])
                nc.sync.dma_start(ccz_in.ap(), zblk[:])
                nc.gpsimd.collective_compute(
                    "ReduceScatter", ALU.add, replica_groups=G8,
                    ins=[ccz_in.ap()], outs=[ccz_out.ap()])
                zfull = l2.tile([BL, D], F32)
                nc.sync.dma_start(zfull[:], ccz_out.ap())
                zb = l2.tile([BL, D], F32)
                nc.sync.dma_start(zb[:], fc1b_in.ap())
                nc.vector.tensor_tensor(zfull[:], zfull[:], zb[:], ALU.add)
                zr = l2.tile([BL, D], F32)
                nc.scalar.activation(zr[:], zfull[:], ACT.Relu)
                if z_dbg is not None:
                    nc.sync.dma_start(z_dbg.ap(), zr[:])

                # fc2 + log_softmax on my 4 batches
                f2w = l2.tile([128, 4, C], BF16)
                nc.sync.dma_start(f2w[:],
                                  fc2w_in.ap().rearrange("(t p) c -> p t c", p=128))
                lps = psl.tile([BL, C], F32)
                for t4 in range(4):
                    ztp = ps2t.tile([128, BL], F32, tag="tr2", name=f"zt_{t4}")
                    nc.tensor.transpose(ztp[:], zr[:, 128 * t4:128 * (t4 + 1)],
                                        identf[:BL, :BL])
                    zts = l2.tile([128, BL], BF16, tag="zts", name=f"zts_{t4}")
                    nc.any.tensor_copy(out=zts[:], in_=ztp[:])
                    nc.tensor.matmul(lps[:], zts[:], f2w[:, t4, :],
                                     start=(t4 == 0), stop=(t4 == 3))
                logits = l2.tile([BL, C], F32)
                f2b = l2.tile([BL, C], F32)
                nc.sync.dma_start(f2b[:], fc2b_in.ap())
                nc.vector.tensor_tensor(logits[:], lps[:], f2b[:], ALU.add)

                mx = l2.tile([BL, 1], F32)
                nc.vector.tensor_reduce(mx[:], logits[:], axis=AX.X, op=ALU.max)
                sh = l2.tile([BL, C], F32)
                nc.vector.tensor_tensor(sh[:], logits[:],
                                        mx[:].to_broadcast((BL, C)), ALU.subtract)
                ex = l2.tile([BL, C], F32)
                nc.scalar.activation(ex[:], sh[:], ACT.Exp)
                sm = l2.tile([BL, 1], F32)
                nc.vector.tensor_reduce(sm[:], ex[:], axis=AX.X, op=ALU.add)
                lg = l2.tile([BL, 1], F32)
                nc.scalar.activation(lg[:], sm[:], ACT.Ln)
                res = l2.tile([BL, C], F32)
                nc.vector.tensor_tensor(res[:], sh[:],
                                        lg[:].to_broadcast((BL, C)), ALU.subtract)
                nc.sync.dma_start(out_t.ap(), res[:])


def _identity_cos():
    t = np.arange(T)
    f = np.arange(T)
    return np.cos(2.0 * np.pi * np.outer(t, f) / T).astype(np.float32)


def make_inputs(x, edge_index0, edge_index2, W1, b1, W2, b2,
                fc1_w, fc1_b, fc2_w, fc2_b):
    """Build the 8 per-core input maps."""
    A0 = _dense_adj(np.asarray(edge_index0), N0)
    A2 = _dense_adj(np.asarray(edge_index2), N2)
    At1 = np.ascontiguousarray((2.0 * A0).T)   # [j, i] = 2*A0[i, j]
    At2 = np.ascontiguousarray((2.0 * A2).T)

    # resident: rows p*NRES+t = At1 row t*128+p
    at_res = _b16(At1[:JRES].reshape(NRES, 128, N0)
                  .transpose(1, 0, 2).reshape(128 * NRES, N0))
    s = At1[JRES:].reshape(NSTR, 128, N0).transpose(1, 0, 2)  # [p, t, n]
    at_str = np.stack([s[:, :, SL * g:SL * (g + 1)] for g in range(NSL)], 0)
    at_str = _b16(at_str.reshape(NSL * 128 * NSTR, SL))
    a2t = _b16(At2.reshape(NT2, 128, N2).transpose(1, 0, 2).reshape(128 * NT2, N2))

    Ccos = _identity_cos()
    W1e = np.einsum("tf,kfg->ktg", Ccos, np.asarray(W1, np.float32))  # [K, 30, G1]
    w1a = np.zeros((128, K, G1), np.float32)
    for bb in range(4):
        w1a[32 * bb:32 * bb + 30] = W1e.transpose(1, 0, 2)
    w1a = _b16(w1a.reshape(128, K * G1))

    W2f = np.asarray(W2, np.float32)       # [K, G1, G2]
    w2a = np.zeros((128, K, 2, G1), np.float32)
    for bb in range(4):
        for hh in range(2):
            w2a[32 * bb:32 * bb + 32, :, hh, :] = \
                W2f[:, :, 32 * hh:32 * hh + 32].transpose(1, 0, 2)
    w2a = _b16(w2a.reshape(128, K * 2 * G1))

    b1v = np.tile(np.asarray(b1, np.float32), 4).reshape(128, 1)
    b2f = np.asarray(b2, np.float32)
    b2v = np.stack([np.tile(b2f[:32], 4), np.tile(b2f[32:], 4)], 1).astype(np.float32)

    fc1b = np.tile(np.asarray(fc1_b, np.float32)[None, :], (BL, 1))
    fc2b = np.tile(np.asarray(fc2_b, np.float32)[None, :], (BL, 1))
    fc2w = _b16(np.asarray(fc2_w, np.float32))

    xf = np.asarray(x, np.float32)          # [B, N0, T]
    fc1wf = np.asarray(fc1_w, np.float32)   # [N2*G2, D]

    ins = []
    for core in range(NCORES):
        xs = xf[BL * core:BL * (core + 1)]          # [4, N0, 30]
        x_cn = np.zeros((BL, TP, N0), np.float32)
        x_cn[:, :T] = xs.transpose(0, 2, 1)
        x_cn = x_cn.reshape(CH, N0)                 # row = b*32 + t
        x_lt = x_cn.T.reshape(NT0, 128, CH).transpose(1, 0, 2).reshape(128 * NT0, CH)
        # fcw[p=n2l, kt=g2, d] = fc1_w[8192*core + n2l*64 + g2, d]: the
        # natural row-major order of the core's slice, no interleave.
        fc1w_r = fc1wf[FBLK * core:FBLK * (core + 1)]
        ins.append({
            "at_res": at_res, "at_str": at_str, "a2t": a2t,
            "x_cn": _b16(x_cn), "x_lt": _b16(x_lt),
            "w1a": w1a, "w2a": w2a, "b1v": b1v, "b2v": b2v,
            "fc1w": _b16(fc1w_r), "fc1b": fc1b,
            "fc2w": fc2w, "fc2b": fc2b,
        })
    return ins


_CACHED = {}


def kernel(**inputs):
    if "nc" not in _CACHED:
        _CACHED["nc"] = build_program(dbg=False)
    nc = _CACHED["nc"]
    ins = make_inputs(**inputs)
    res = run_bass_kernel_spmd(nc, ins, core_ids=list(range(NCORES)))
    out = np.zeros((B, C), np.float32)
    for core in range(NCORES):
        out[BL * core:BL * (core + 1)] = res.results[core]["out"]
    return out


# revision 22
# speedup vs baseline: 1.0358x; 1.0060x over previous
"""NetTGCN forward pass on 8 Trainium2 NeuronCores (Bass/Tile).

Batch-parallel design, zero collectives until the fc head:
  Each core owns 4 batches. Layer-1 channels = 4 batches x 32 taps = 128 =
  exactly the SBUF partition width, so the full Chebyshev recurrence on the
  4096-node graph runs locally per core: state kept in SBUF in both
  [ch, node] (recurrence/contract) and node-major lhsT form (matmul
  stationary). The dense operator 2A^T (bf16, 33.5 MB) is split: 14 of 32
  contract row-tiles stay SBUF-resident, the other 18 are streamed from HBM
  per 512-column output slice (2.4 MB contiguous DMAs, hidden under the
  matmuls). Per Chebyshev term: 256 matmuls of [128x128]@[128x512] (~99% PE
  eff), 32 PE transposes to rebuild the lhsT form, and an inline W1[k]
  contraction into the fp32 h1 accumulator.
  The FFT is folded into W1 on the host (real(FFT(x)) = x @ Ccos commutes
  with the graph operator).
  Layer 2 (1024-node graph) is identical in structure with the 2 MB
  operator fully resident.
  Head: features are exchanged with one 8-rank AllToAll so each core
  contracts its 8192-row slice of fc1_w for all 32 batches; partial z is
  ReduceScattered (each core gets its own 4 batches), fc2 + log_softmax run
  locally, and the host concatenates per-core outputs.

States are bf16 throughout (validated on host: final rel err 6.4e-3 vs
6.1e-3 for fp32 states); accumulators (h1/h2/psum) are fp32.
"""

import sys

if "/opt/trn_rl_repo" not in sys.path:
    sys.path.insert(0, "/opt/trn_rl_repo")

import numpy as np
import ml_dtypes

import concourse.bacc as bacc
import concourse.mybir as mybir
import concourse.bass_utils as _bu
from concourse.bass_utils import run_bass_kernel_spmd
from concourse.tile import TileContext
from concourse.masks import make_identity

_bu.upload_artifacts = lambda tmpdir: f"file://{tmpdir}"  # no bucket in sandbox

F32 = mybir.dt.float32
BF16 = mybir.dt.bfloat16
AX = mybir.AxisListType
ALU = mybir.AluOpType
ACT = mybir.ActivationFunctionType

B, N0, T, K = 32, 4096, 30, 25
G1, G2, D, C = 32, 64, 512, 10
N2 = N0 // 4
NCORES = 8
BL = B // NCORES       # 4 batches per core
TP = 32                # taps padded 30 -> 32
CH = BL * TP           # 128 layer-1 channels = partition width
NT0 = N0 // 128        # 32 contract tiles (layer 1)
NRES = 14              # operator row-tiles resident in SBUF
NSTR = NT0 - NRES      # 19 streamed row-tiles
JRES = NRES * 128
SL = 512               # output slice width
NSL = N0 // SL         # 8 slices per term
NT2 = N2 // 128        # 8 contract tiles (layer 2)
FBLK = (N2 * G2) // NCORES  # 8192 fc1 contraction rows per core

G8 = [list(range(NCORES))]


def _b16(a):
    return np.ascontiguousarray(a.astype(ml_dtypes.bfloat16))


def _dense_adj(edge_index, n):
    row = edge_index[0].astype(np.int64)
    col = edge_index[1].astype(np.int64)
    deg = np.zeros(n, np.float32)
    np.add.at(deg, row, 1.0)
    dis = np.where(deg > 0, 1.0 / np.sqrt(np.maximum(deg, 1.0)), 0.0).astype(np.float32)
    w = (-dis[row] * dis[col]).astype(np.float32)
    a = np.zeros((n, n), np.float32)
    np.add.at(a, (row, col), w)
    return a


def build_program(dbg=False):
    nc = bacc.Bacc("TRN2", target_bir_lowering=False, debug=False,
                   num_devices=NCORES)

    at_res_in = nc.dram_tensor("at_res", [128 * NRES, N0], BF16, kind="ExternalInput")
    at_str_in = nc.dram_tensor("at_str", [NSL * 128 * NSTR, SL], BF16, kind="ExternalInput")
    x_cn_in = nc.dram_tensor("x_cn", [128, N0], BF16, kind="ExternalInput")
    x_lt_in = nc.dram_tensor("x_lt", [128 * NT0, CH], BF16, kind="ExternalInput")
    w1_in = nc.dram_tensor("w1a", [128, K * G1], BF16, kind="ExternalInput")
    b1_in = nc.dram_tensor("b1v", [128, 1], F32, kind="ExternalInput")
    a2t_in = nc.dram_tensor("a2t", [128 * NT2, N2], BF16, kind="ExternalInput")
    w2_in = nc.dram_tensor("w2a", [128, K * 2 * G1], BF16, kind="ExternalInput")
    b2_in = nc.dram_tensor("b2v", [128, 2], F32, kind="ExternalInput")
    fc1w_in = nc.dram_tensor("fc1w", [128 * (FBLK // 128), D], BF16, kind="ExternalInput")
    fc1b_in = nc.dram_tensor("fc1b", [BL, D], F32, kind="ExternalInput")
    fc2w_in = nc.dram_tensor("fc2w", [D, C], BF16, kind="ExternalInput")
    fc2b_in = nc.dram_tensor("fc2b", [BL, C], F32, kind="ExternalInput")

    out_t = nc.dram_tensor("out", [BL, C], F32, kind="ExternalOutput")
    if dbg:
        h1_dbg = nc.dram_tensor("h1_dbg", [128, N0], F32, kind="ExternalOutput")
        h1p_dbg = nc.dram_tensor("h1p_dbg", [128, N2], F32, kind="ExternalOutput")
        h2_dbg = nc.dram_tensor("h2_dbg", [128, 2 * N2], F32, kind="ExternalOutput")
        z_dbg = nc.dram_tensor("z_dbg", [BL, D], F32, kind="ExternalOutput")

    cch_in = nc.dram_tensor("cch_in", [NCORES * 128 * 64, BL], BF16)
    cch_out = nc.dram_tensor("cch_out", [NCORES * 128 * 64, BL], BF16)
    ccz_in = nc.dram_tensor("ccz_in", [B, D], F32)
    ccz_out = nc.dram_tensor("ccz_out", [BL, D], F32)

    with TileContext(nc) as tc:
        with tc.tile_pool(name="const", bufs=1) as cpool:
            identb = cpool.tile([128, 128], BF16)
            make_identity(nc, identb[:])
            identf = cpool.tile([128, 128], F32)
            make_identity(nc, identf[:])
            h1_sb = cpool.tile([128, N0], F32)
            h1p = cpool.tile([128, N2], F32)

            # ======================= LAYER 1 =======================
            with tc.tile_pool(name="l1a", bufs=1) as l1a, \
                 tc.tile_pool(name="l1s", bufs=4) as l1s, \
                 tc.tile_pool(name="l1st", bufs=1) as l1st, \
                 tc.tile_pool(name="ps_y", bufs=2, space="PSUM") as ps_y, \
                 tc.tile_pool(name="ps_tr", bufs=4, space="PSUM") as ps_tr, \
                 tc.tile_pool(name="ps_h", bufs=2, space="PSUM") as ps_h:

                # small inputs first: the DMA rings are FIFO, so the x /
                # weight loads must not queue behind 13 MB of operator tiles
                w1a = l1a.tile([128, K, G1], BF16)
                nc.sync.dma_start(w1a[:], w1_in.ap().rearrange("p (k g) -> p k g", k=K))
                b1v = l1a.tile([128, 1], F32)
                nc.sync.dma_start(b1v[:], b1_in.ap())
                cn = [l1st.tile([128, N0], BF16, name=f"cn{i}", tag=f"cn{i}")
                      for i in range(2)]
                lt = [l1st.tile([128, NT0, CH], BF16, name=f"lt{i}", tag=f"lt{i}")
                      for i in range(2)]
                nc.sync.dma_start(cn[0][:], x_cn_in.ap())
                nc.sync.dma_start(lt[0][:],
                                  x_lt_in.ap().rearrange("(p t) c -> p t c", t=NT0))

                at_res = l1a.tile([128, NRES, N0], BF16)
                at_res_v = at_res_in.ap().rearrange("(p t) n -> p t n", t=NRES)
                for t in range(NRES):
                    nc.sync.dma_start(at_res[:, t, :], at_res_v[:, t, :])

                at_str_v = at_str_in.ap().rearrange(
                    "(g p t) n -> g p t n", g=NSL, p=128)

                def contract1(src_cn, kk, g, first):
                    sl = slice(SL * g, SL * (g + 1))
                    hp = ps_h.tile([128, SL], F32, tag="hp", name=f"hp{kk}_{g}")
                    for bb in range(4):
                        nc.tensor.matmul(
                            hp[32 * bb:32 * (bb + 1), :],
                            w1a[32 * bb:32 * (bb + 1), kk, :],
                            src_cn[32 * bb:32 * (bb + 1), sl],
                            start=True, stop=True,
                            tile_position=(32 * bb, 32 * bb))
                    if first:
                        nc.vector.tensor_copy(h1_sb[:, sl], hp[:])
                    else:
                        nc.vector.tensor_tensor(h1_sb[:, sl], h1_sb[:, sl],
                                                hp[:], ALU.add)

                for g in range(NSL):
                    contract1(cn[0], 0, g, True)

                def epi1(k, g):
                    """Transposes + h1 contract for slice (k, g); emitted
                    after the NEXT slice's matmuls so PE never stalls on
                    the DVE recurrence."""
                    new_cn = cn[k % 2]
                    new_lt = lt[k % 2]
                    for bq in range(4):
                        trp = ps_tr.tile([128, 128], BF16, tag="trp",
                                         name=f"trp{k}_{g}_{bq}")
                        nc.tensor.transpose(
                            trp[:],
                            new_cn[:, SL * g + 128 * bq:SL * g + 128 * (bq + 1)],
                            identb[:])
                        nc.vector.tensor_copy(new_lt[:, 4 * g + bq, :], trp[:])
                    contract1(new_cn, k, g, False)

                pend = None
                for k in range(1, K):
                    cur_lt = lt[(k - 1) % 2]
                    # in-place ring: tx_k overwrites tx_{k-2} slice by slice
                    new_cn = cn[k % 2]
                    prev_cn = new_cn if k >= 2 else None
                    for g in range(NSL):
                        sl = slice(SL * g, SL * (g + 1))
                        sa = l1s.tile([128, 9, SL], BF16, tag="sa",
                                      name=f"sa{k}_{g}")
                        sb = l1s.tile([128, 9, SL], BF16, tag="sa",
                                      name=f"sb{k}_{g}")
                        nc.sync.dma_start(sa[:], at_str_v[g][:, 0:9, :])
                        nc.sync.dma_start(sb[:], at_str_v[g][:, 9:18, :])
                        yp = ps_y.tile([128, SL], F32, tag="yp", name=f"yp{k}_{g}")
                        for jt in range(NRES):
                            nc.tensor.matmul(yp[:], cur_lt[:, jt, :],
                                             at_res[:, jt, sl],
                                             start=(jt == 0), stop=False)
                        # previous slice's epilogue sits inside this MM
                        # group: its lhsT tiles are only read by the later
                        # streamed matmuls (t >= 14), so PE never stalls.
                        if pend is not None:
                            pend()
                            pend = None
                        for t in range(9):
                            nc.tensor.matmul(yp[:], cur_lt[:, NRES + t, :],
                                             sa[:, t, :],
                                             start=False, stop=False)
                        for t in range(9):
                            nc.tensor.matmul(yp[:], cur_lt[:, NRES + 9 + t, :],
                                             sb[:, t, :],
                                             start=False, stop=(t == 8))
                        if k == 1:
                            nc.vector.tensor_scalar_mul(new_cn[:, sl], yp[:], 0.5)
                        else:
                            nc.vector.tensor_tensor(new_cn[:, sl], yp[:],
                                                    prev_cn[:, sl], ALU.subtract)
                        pend = (lambda kk=k, gg=g: epi1(kk, gg))
                pend()
                pend = None

                # bias + relu + maxpool4 along nodes
                for q in range(4):
                    nc.scalar.activation(h1_sb[:, 1024 * q:1024 * (q + 1)],
                                         h1_sb[:, 1024 * q:1024 * (q + 1)],
                                         ACT.Relu, bias=b1v[:])
                if dbg:
                    nc.sync.dma_start(h1_dbg.ap(), h1_sb[:])
                h4 = h1_sb[:].rearrange("p (n f) -> p n f", f=4)
                nc.vector.tensor_tensor(h1p[:], h4[:, :, 0], h4[:, :, 1], ALU.max)
                nc.vector.tensor_tensor(h1p[:], h1p[:], h4[:, :, 2], ALU.max)
                nc.vector.tensor_tensor(h1p[:], h1p[:], h4[:, :, 3], ALU.max)
                if dbg:
                    nc.sync.dma_start(h1p_dbg.ap(), h1p[:])

            # ======================= LAYER 2 =======================
            with tc.tile_pool(name="l2", bufs=1) as l2, \
                 tc.tile_pool(name="l2st", bufs=1) as l2st, \
                 tc.tile_pool(name="ps2t", bufs=2, space="PSUM") as ps2t:

                a2t = l2.tile([128, NT2, N2], BF16)
                a2t_v = a2t_in.ap().rearrange("(p t) n -> p t n", t=NT2)
                for t in range(NT2):
                    nc.sync.dma_start(a2t[:, t, :], a2t_v[:, t, :])
                w2a = l2.tile([128, K, 2, G1], BF16)
                nc.sync.dma_start(
                    w2a[:], w2_in.ap().rearrange("p (k h g) -> p k h g", k=K, h=2))
                b2v = l2.tile([128, 2], F32)
                nc.sync.dma_start(b2v[:], b2_in.ap())
                fcw = l2.tile([128, FBLK // 128, D], BF16)
                fcw_v = fc1w_in.ap().rearrange("(p t) d -> p t d", t=FBLK // 128)
                for q in range(8):
                    nc.sync.dma_start(fcw[:, 8 * q:8 * (q + 1), :],
                                      fcw_v[:, 8 * q:8 * (q + 1), :])

                cn2 = [l2st.tile([128, N2], BF16, name=f"cn2_{i}", tag=f"cn2_{i}")
                       for i in range(3)]
                lt2 = [l2st.tile([128, NT2, CH], BF16, name=f"lt2_{i}",
                                 tag=f"lt2_{i}") for i in range(2)]
                h2r = l2.tile([128, 2, N2], F32)
                ft = l2.tile([128, NT2, G2, BL], BF16)

                with tc.tile_pool(name="ps2y", bufs=2, space="PSUM") as ps2y, \
                     tc.tile_pool(name="ps2h", bufs=1, space="PSUM") as ps2h:
                    # h2 accumulates in PSUM across all K terms: 4 banks,
                    # start at k=0, stop at k=K-1, relu reads PSUM directly.
                    h2ps = [[ps2h.tile([128, SL], F32, tag=f"h2ps_{hh}_{g}",
                                       name=f"h2ps_{hh}_{g}")
                             for g in range(2)] for hh in range(2)]

                    nc.vector.tensor_copy(cn2[0][:], h1p[:])
                    for nt in range(NT2):
                        trp = ps2t.tile([128, 128], BF16, tag="tr2",
                                        name=f"tr2i_{nt}")
                        nc.tensor.transpose(
                            trp[:], cn2[0][:, 128 * nt:128 * (nt + 1)], identb[:])
                        nc.vector.tensor_copy(lt2[0][:, nt, :], trp[:])

                    def contract2(src_cn, kk, g):
                        sl = slice(SL * g, SL * (g + 1))
                        for hh in range(2):
                            for bb in range(4):
                                nc.tensor.matmul(
                                    h2ps[hh][g][32 * bb:32 * (bb + 1), :],
                                    w2a[32 * bb:32 * (bb + 1), kk, hh, :],
                                    src_cn[32 * bb:32 * (bb + 1), sl],
                                    start=(kk == 0), stop=(kk == K - 1),
                                    tile_position=(32 * bb, 32 * bb))

                    def epi2(k, g):
                        new_cn2 = cn2[k % 3]
                        new_lt2 = lt2[k % 2]
                        for bq in range(4):
                            trp = ps2t.tile([128, 128], BF16, tag="tr2",
                                            name=f"tr2_{k}_{g}_{bq}")
                            nc.tensor.transpose(
                                trp[:],
                                new_cn2[:, SL * g + 128 * bq:SL * g + 128 * (bq + 1)],
                                identb[:])
                            nc.vector.tensor_copy(new_lt2[:, 4 * g + bq, :], trp[:])
                        contract2(new_cn2, k, g)

                    contract2(cn2[0], 0, 0)
                    contract2(cn2[0], 0, 1)
                    pend2 = None
                    for k in range(1, K):
                        cur_lt2 = lt2[(k - 1) % 2]
                        new_cn2 = cn2[k % 3]
                        prev_cn2 = cn2[(k - 2) % 3] if k >= 2 else None
                        for g in range(2):
                            sl = slice(SL * g, SL * (g + 1))
                            y2 = ps2y.tile([128, SL], F32, tag="y2",
                                           name=f"y2_{k}_{g}")
                            for jt in range(4):
                                nc.tensor.matmul(y2[:], cur_lt2[:, jt, :],
                                                 a2t[:, jt, sl],
                                                 start=(jt == 0), stop=False)
                            if pend2 is not None:
                                pend2()
                                pend2 = None
                            for jt in range(4, NT2):
                                nc.tensor.matmul(y2[:], cur_lt2[:, jt, :],
                                                 a2t[:, jt, sl],
                                                 start=False,
                                                 stop=(jt == NT2 - 1))
                            if k == 1:
                                nc.vector.tensor_scalar_mul(new_cn2[:, sl],
                                                            y2[:], 0.5)
                            else:
                                nc.vector.tensor_tensor(new_cn2[:, sl], y2[:],
                                                        prev_cn2[:, sl],
                                                        ALU.subtract)
                            pend2 = (lambda kk=k, gg=g: epi2(kk, gg))
                    pend2()
                    pend2 = None

                    # ================== HEAD (part 1) ==================
                    for hh in range(2):
                        for g in range(2):
                            sl = slice(SL * g, SL * (g + 1))
                            nc.scalar.activation(h2r[:, hh, sl], h2ps[hh][g][:],
                                                 ACT.Relu, bias=b2v[:, hh:hh + 1])
                    if dbg:
                        nc.sync.dma_start(
                            h2_dbg.ap().rearrange("p (h n) -> p h n", h=2),
                            h2r[:])

                    # features to f-major: ft[n2l, nt, g2, b]
                    for hh in range(2):
                        for nt in range(NT2):
                            trp = ps2t.tile([128, 128], F32, tag="tr2",
                                            name=f"trh_{hh}_{nt}")
                            nc.tensor.transpose(
                                trp[:], h2r[:, hh, 128 * nt:128 * (nt + 1)],
                                identf[:])
                            nc.vector.tensor_copy(
                                out=ft[:, nt, 32 * hh:32 * (hh + 1), :],
                                in_=trp[:].rearrange("p (b g) -> p g b", b=BL))
                    nc.sync.dma_start(
                        cch_in.ap().rearrange("(j nl g) b -> nl j (g b)",
                                              j=NCORES, nl=128),
                        ft[:])
                    nc.gpsimd.collective_compute(
                        "AllToAll", ALU.bypass, replica_groups=G8,
                        ins=[cch_in.ap()], outs=[cch_out.ap()])

                # ================== HEAD (part 2) ==================
                # fc1 partial: z[32, D] for my f-slice. flt is
                # [p=n2l, r, q=g2, b] so each per-rank DMA from cch_out is
                # contiguous on both sides; the matmul stationary reads the
                # strided [p, (r, b)] AP per g2.
                with tc.tile_pool(name="psz", bufs=1, space="PSUM") as psz, \
                     tc.tile_pool(name="psl", bufs=1, space="PSUM") as psl:
                    head_tail(nc, tc, l2, ps2t, psz, psl, identf,
                              cch_out, ccz_in, ccz_out, fcw,
                              fc1b_in, fc2w_in, fc2b_in, out_t,
                              z_dbg if dbg else None)

    nc.compile()
    return nc


def head_tail(nc, tc, l2, ps2t, psz, psl, identf, cch_out, ccz_in, ccz_out,
              fcw, fc1b_in, fc2w_in, fc2b_in, out_t, z_dbg):
    """fc1 partial + ReduceScatter + fc2 + log_softmax."""
    if True:
        if True:
            if True:
                flt = l2.tile([128, NCORES, G2, BL], BF16)
                cch_v = cch_out.ap().rearrange(
                    "(r p q) b -> r p q b", r=NCORES, p=128)
                for r in range(NCORES):
                    nc.sync.dma_start(flt[:, r, :, :], cch_v[r])
                flt2 = l2.tile([128, G2, B], BF16)
                nc.vector.tensor_copy(
                    out=flt2[:].rearrange("p q (r b) -> p q r b", r=NCORES),
                    in_=flt[:].rearrange("p r q b -> p q r b"))
                zps = psz.tile([32, D], F32)
                for kt in range(G2):
                    nc.tensor.matmul(zps[:], flt2[:, kt, :], fcw[:, kt, :],
                                     start=(kt == 0), stop=(kt == G2 - 1))
                zblk = l2.tile([32, D], F32)
                nc.vector.tensor_copy(zblk[:], zps[:])
                nc.sync.dma_start(ccz_in.ap(), zblk[:])
                nc.gpsimd.collective_compute(
                    "ReduceScatter", ALU.add, replica_groups=G8,
                    ins=[ccz_in.ap()], outs=[ccz_out.ap()])
                zfull = l2.tile([BL, D], F32)
                nc.sync.dma_start(zfull[:], ccz_out.ap())
                zb = l2.tile([BL, D], F32)
                nc.sync.dma_start(zb[:], fc1b_in.ap())
                nc.vector.tensor_tensor(zfull[:], zfull[:], zb[:], ALU.add)
                zr = l2.tile([BL, D], F32)
                nc.scalar.activation(zr[:], zfull[:], ACT.Relu)
                if z_dbg is not None:
                    nc.sync.dma_start(z_dbg.ap(), zr[:])

                # fc2 + log_softmax on my 4 batches
                f2w = l2.tile([128, 4, C], BF16)
                nc.sync.dma_start(f2w[:],
                                  fc2w_in.ap().rearrange("(t p) c -> p t c", p=128))
                lps = psl.tile([BL, C], F32)
                for t4 in range(4):
                    ztp = ps2t.tile([128, BL], F32, tag="tr2", name=f"zt_{t4}")
                    nc.tensor.transpose(ztp[:], zr[:, 128 * t4:128 * (t4 + 1)],
                                        identf[:BL, :BL])
                    zts = l2.tile([128, BL], BF16, tag="zts", name=f"zts_{t4}")
                    nc.any.tensor_copy(out=zts[:], in_=ztp[:])
                    nc.tensor.matmul(lps[:], zts[:], f2w[:, t4, :],
                                     start=(t4 == 0), stop=(t4 == 3))
                logits = l2.tile([BL, C], F32)
                f2b = l2.tile([BL, C], F32)
                nc.sync.dma_start(f2b[:], fc2b_in.ap())
                nc.vector.tensor_tensor(logits[:], lps[:], f2b[:], ALU.add)

                mx = l2.tile([BL, 1], F32)
                nc.vector.tensor_reduce(mx[:], logits[:], axis=AX.X, op=ALU.max)
                sh = l2.tile([BL, C], F32)
                nc.vector.tensor_tensor(sh[:], logits[:],
                                        mx[:].to_broadcast((BL, C)), ALU.subtract)
                ex = l2.tile([BL, C], F32)
                nc.scalar.activation(ex[:], sh[:], ACT.Exp)
                sm = l2.tile([BL, 1], F32)
                nc.vector.tensor_reduce(sm[:], ex[:], axis=AX.X, op=ALU.add)
                lg = l2.tile([BL, 1], F32)
                nc.scalar.activation(lg[:], sm[:], ACT.Ln)
                res = l2.tile([BL, C], F32)
                nc.vector.tensor_tensor(res[:], sh[:],
                                        lg[:].to_broadcast((BL, C)), ALU.subtract)
                nc.sync.dma_start(out_t.ap(), res[:])


def _identity_cos():
    t = np.arange(T)
    f = np.arange(T)
    return np.cos(2.0 * np.pi * np.outer(t, f) / T).astype(np.float32)


def make_inputs(x, edge_index0, edge_index2, W1, b1, W2, b2,
                fc1_w, fc1_b, fc2_w, fc2_b):
    """Build the 8 per-core input maps."""
    A0 = _dense_adj(np.asarray(edge_index0), N0)
    A2 = _dense_adj(np.asarray(edge_index2), N2)
    At1 = np.ascontiguousarray((2.0 * A0).T)   # [j, i] = 2*A0[i, j]
    At2 = np.ascontiguousarray((2.0 * A2).T)

    # resident: rows p*NRES+t = At1 row t*128+p
    at_res = _b16(At1[:JRES].reshape(NRES, 128, N0)
                  .transpose(1, 0, 2).reshape(128 * NRES, N0))
    s = At1[JRES:].reshape(NSTR, 128, N0).transpose(1, 0, 2)  # [p, t, n]
    at_str = np.stack([s[:, :, SL * g:SL * (g + 1)] for g in range(NSL)], 0)
    at_str = _b16(at_str.reshape(NSL * 128 * NSTR, SL))
    a2t = _b16(At2.reshape(NT2, 128, N2).transpose(1, 0, 2).reshape(128 * NT2, N2))

    Ccos = _identity_cos()
    W1e = np.einsum("tf,kfg->ktg", Ccos, np.asarray(W1, np.float32))  # [K, 30, G1]
    w1a = np.zeros((128, K, G1), np.float32)
    for bb in range(4):
        w1a[32 * bb:32 * bb + 30] = W1e.transpose(1, 0, 2)
    w1a = _b16(w1a.reshape(128, K * G1))

    W2f = np.asarray(W2, np.float32)       # [K, G1, G2]
    w2a = np.zeros((128, K, 2, G1), np.float32)
    for bb in range(4):
        for hh in range(2):
            w2a[32 * bb:32 * bb + 32, :, hh, :] = \
                W2f[:, :, 32 * hh:32 * hh + 32].transpose(1, 0, 2)
    w2a = _b16(w2a.reshape(128, K * 2 * G1))

    b1v = np.tile(np.asarray(b1, np.float32), 4).reshape(128, 1)
    b2f = np.asarray(b2, np.float32)
    b2v = np.stack([np.tile(b2f[:32], 4), np.tile(b2f[32:], 4)], 1).astype(np.float32)

    fc1b = np.tile(np.asarray(fc1_b, np.float32)[None, :], (BL, 1))
    fc2b = np.tile(np.asarray(fc2_b, np.float32)[None, :], (BL, 1))
    fc2w = _b16(np.asarray(fc2_w, np.float32))

    xf = np.asarray(x, np.float32)          # [B, N0, T]
    fc1wf = np.asarray(fc1_w, np.float32)   # [N2*G2, D]

    ins = []
    for core in range(NCORES):
        xs = xf[BL * core:BL * (core + 1)]          # [4, N0, 30]
        x_cn = np.zeros((BL, TP, N0), np.float32)
        x_cn[:, :T] = xs.transpose(0, 2, 1)
        x_cn = x_cn.reshape(CH, N0)                 # row = b*32 + t
        x_lt = x_cn.T.reshape(NT0, 128, CH).transpose(1, 0, 2).reshape(128 * NT0, CH)
        # fcw[p=n2l, kt=g2, d] = fc1_w[8192*core + n2l*64 + g2, d]: the
        # natural row-major order of the core's slice, no interleave.
        fc1w_r = fc1wf[FBLK * core:FBLK * (core + 1)]
        ins.append({
            "at_res": at_res, "at_str": at_str, "a2t": a2t,
            "x_cn": _b16(x_cn), "x_lt": _b16(x_lt),
            "w1a": w1a, "w2a": w2a, "b1v": b1v, "b2v": b2v,
            "fc1w": _b16(fc1w_r), "fc1b": fc1b,
            "fc2w": fc2w, "fc2b": fc2b,
        })
    return ins


_CACHED = {}


def kernel(**inputs):
    if "nc" not in _CACHED:
        _CACHED["nc"] = build_program(dbg=False)
    nc = _CACHED["nc"]
    ins = make_inputs(**inputs)
    res = run_bass_kernel_spmd(nc, ins, core_ids=list(range(NCORES)))
    out = np.zeros((B, C), np.float32)
    for core in range(NCORES):
        out[BL * core:BL * (core + 1)] = res.results[core]["out"]
    return out
